# revision 16
# baseline (speedup 1.0000x reference)
"""Trainium2 Bass kernel for nn_CICDM_Net (cognitive-diagnosis style model).

Data-parallel over 1024 students across 8 NeuronCores (128/core),
parameters replicated. Per core (v2 — matmul-contraction formulation):

  * Count/score matrices M[e, b] = sum_l [exer[b,l]=e] * {1, score[b,l]}
    are built on device: per (student, half-sequence) a bf16 one-hot
    outer product over the split index e = lo*128 + hi (hi = e%128 on
    partitions, lo = e//128 as PSUM free columns), evacuated into
    M_all[e%128, (e//128, {cnt,score}, b)].
  * W = sigmoid(exer_conc_w)*adj is built tile-by-tile (fused row-sum);
    each W tile feeds (a) TensorE transposes -> W^T bounced via HBM for
    the Y_A matmul, (b) dense matmuls lhsT=W-tile x rhs=M-slice
    accumulating s^T,num^T = [c, (s|num, b)] over all 64 e-tiles, and
    (c) lhsT=exp(pote)-tile for Z^T,V^T (potential-softmax sums).
  * a = num/s (masked), A = (a@e)/(mask@e), e = exp(conc_conc_w)
    (softmax shifts dropped: value ranges make exp exact-safe).
  * Y = q0*clip((1-lam)/rowsum*(A@W^T) + lam*(Bm@D2L^T)) + q1, with
    per-exercise coefficient rows DMA-broadcast across partitions.

All matmuls are fp32 (float32r is reduced precision - measured 1.6e-4)
except the one-hot outer products, which are exact in bf16.
"""

import os
import sys

import numpy as np

for _p in ("/opt/trn_rl_repo", "/root/.axon_site/_ro/trn_rl_repo"):
    if os.path.isdir(_p) and _p not in sys.path:
        sys.path.append(_p)

from contextlib import ExitStack

import concourse.bass as bass
import concourse.tile as tile
from concourse import mybir
from concourse.bass_utils import run_bass_kernel_spmd
from concourse.masks import make_identity

FP = mybir.dt.float32
BF = mybir.dt.bfloat16
I32 = mybir.dt.int32
ALU = mybir.AluOpType
ACT = mybir.ActivationFunctionType

E, C, P, L, B = 8192, 512, 32, 256, 1024
NCORES = 8
BS = B // NCORES          # 128 students per core
ET = E // 128             # 64 exercise tiles
CT = C // 128             # 4 concept tiles
ECH = E // 512            # 16 output chunks


def _emit(ctx: ExitStack, tc: tile.TileContext):
    nc = tc.nc

    x_in = nc.dram_tensor("exer_list", (BS, L), I32, kind="ExternalInput").ap()
    sc_in = nc.dram_tensor("score_list", (BS, L), FP, kind="ExternalInput").ap()
    adj_in = nc.dram_tensor("exer_conc_adj", (E, C), FP, kind="ExternalInput").ap()
    ecw_in = nc.dram_tensor("exer_conc_w", (E, C), FP, kind="ExternalInput").ap()
    cc_in = nc.dram_tensor("conc_conc_w", (C, C), FP, kind="ExternalInput").ap()
    pote_in = nc.dram_tensor("exer_pote_w", (E, P), FP, kind="ExternalInput").ap()
    lam_in = nc.dram_tensor("lambd", (1, E), FP, kind="ExternalInput").ap()
    gue_in = nc.dram_tensor("guess", (1, E), FP, kind="ExternalInput").ap()
    sli_in = nc.dram_tensor("slide", (1, E), FP, kind="ExternalInput").ap()
    a_out = nc.dram_tensor("A_out", (BS, C), FP, kind="ExternalOutput").ap()
    y_out = nc.dram_tensor("Y_out", (BS, E), FP, kind="ExternalOutput").ap()

    dram = ctx.enter_context(tc.tile_pool(name="dram", bufs=1, space="DRAM"))
    wth = dram.tile([C, E], FP, tag="wth", name="wth")[:]
    d2lth = dram.tile([P, E], FP, tag="d2lth", name="d2lth")[:]
    c1h = dram.tile([1, E], FP, tag="c1h", name="c1h")[:]
    q0h = dram.tile([1, E], FP, tag="q0h", name="q0h")[:]
    q1h = dram.tile([1, E], FP, tag="q1h", name="q1h")[:]

    res = ctx.enter_context(tc.tile_pool(name="res", bufs=1))
    scr = ctx.enter_context(tc.tile_pool(name="scr", bufs=2))
    stream = ctx.enter_context(tc.tile_pool(name="stream", bufs=4))
    oh = ctx.enter_context(tc.tile_pool(name="oh", bufs=4))
    ypool = ctx.enter_context(tc.tile_pool(name="ypool", bufs=2))
    rows = ctx.enter_context(tc.tile_pool(name="rows", bufs=2))
    ptr = ctx.enter_context(tc.tile_pool(name="ptr", bufs=2, space="PSUM"))

    # ---------- small/shared prep ----------
    ident = res.tile([128, 128], FP, tag="ident", name="ident")[:]
    make_identity(nc, ident)
    iota128 = res.tile([128, 128], FP, tag="iota128", name="iota128")[:]
    it128 = scr.tile([128, 128], I32, tag="it128", name="it128")[:]
    nc.gpsimd.iota(it128, pattern=[[1, 128]], base=0, channel_multiplier=0)
    nc.vector.tensor_copy(out=iota128, in_=it128)
    iota64 = res.tile([128, 64], FP, tag="iota64", name="iota64")[:]
    nc.vector.tensor_copy(out=iota64, in_=it128[:, 0:64])

    x_sb = stream.tile([BS, L], I32, tag="x", name="x")[:]
    nc.sync.dma_start(out=x_sb, in_=x_in)
    sc_sb = stream.tile([BS, L], FP, tag="sc", name="sc")[:]
    nc.sync.dma_start(out=sc_sb, in_=sc_in)

    # transposed per-half-sequence tiles: hi = x%128, lo = x//128 as f32,
    # score^T; all [l%128, b]
    xf = stream.tile([BS, L], FP, tag="xf", name="xf")[:]
    nc.vector.tensor_copy(out=xf, in_=x_sb)
    hiT, loT, scT = [], [], []
    for j in range(2):
        tpx = ptr.tile([128, 128], FP, tag="tr", name="tr")[:]
        nc.tensor.transpose(tpx, xf[:, 128 * j : 128 * (j + 1)], ident)
        xti = scr.tile([128, 128], I32, tag="xti", name="xti")[:]
        nc.vector.tensor_copy(out=xti, in_=tpx)
        hii = scr.tile([128, 128], I32, tag="hii", name="hii")[:]
        nc.vector.tensor_scalar(
            out=hii, in0=xti, scalar1=127, scalar2=None, op0=ALU.bitwise_and
        )
        hif = res.tile([128, 128], FP, tag=f"hif{j}", name=f"hif{j}")[:]
        nc.vector.tensor_copy(out=hif, in_=hii)
        hiT.append(hif)
        loi = scr.tile([128, 128], I32, tag="loi", name="loi")[:]
        nc.vector.tensor_scalar(
            out=loi, in0=xti, scalar1=7, scalar2=None, op0=ALU.logical_shift_right
        )
        lof = res.tile([128, 128], FP, tag=f"lof{j}", name=f"lof{j}")[:]
        nc.vector.tensor_copy(out=lof, in_=loi)
        loT.append(lof)
        tps = ptr.tile([128, 128], FP, tag="tr", name="tr")[:]
        nc.tensor.transpose(tps, sc_sb[:, 128 * j : 128 * (j + 1)], ident)
        sct = res.tile([128, 128], FP, tag=f"sct{j}", name=f"sct{j}")[:]
        nc.vector.tensor_copy(out=sct, in_=tps)
        scT.append(sct)

    # ---------- M build: one-hot outer products, 1 student per PSUM ----------
    # M_all[p, (t, half, b)] = M[e = t*128 + p, b], half 0 = count, 1 = score
    m_all = res.tile([128, ET * 2 * BS], BF, tag="m_all", name="m_all")[:]
    m4 = m_all.rearrange("p (t h b) -> p t h b", h=2, b=BS)
    with tc.tile_pool(name="pmb", bufs=2, space="PSUM") as pmb:
        for b in range(BS):
            pm = pmb.tile([128, 128], FP, tag="pm", name="pm")[:]
            for j in range(2):
                ohh = oh.tile([128, 128], BF, tag="ohh", name="ohh")[:]
                nc.gpsimd.tensor_scalar(
                    out=ohh,
                    in0=iota128,
                    scalar1=hiT[j][:, b : b + 1],
                    scalar2=None,
                    op0=ALU.is_equal,
                )
                rhsb = oh.tile([128, 128], BF, tag="rhsb", name="rhsb")[:]
                nc.vector.tensor_scalar(
                    out=rhsb[:, 0:64],
                    in0=iota64,
                    scalar1=loT[j][:, b : b + 1],
                    scalar2=None,
                    op0=ALU.is_equal,
                )
                nc.scalar.activation(
                    out=rhsb[:, 64:128],
                    in_=rhsb[:, 0:64],
                    func=ACT.Copy,
                    scale=scT[j][:, b : b + 1],
                )
                nc.tensor.matmul(
                    out=pm, lhsT=ohh, rhs=rhsb, start=(j == 0), stop=(j == 1)
                )
            nc.vector.tensor_copy(out=m4[:, :, 0, b], in_=pm[:, 0:64])
            nc.vector.tensor_copy(out=m4[:, :, 1, b], in_=pm[:, 64:128])

    # ---------- exp(pote), D2 scale, coefficient rows ----------
    # ep layout: [p, t, f] with exercise e = 128*t + p
    ep = res.tile([128, ET * P], FP, tag="ep", name="ep")[:]
    ep3 = ep.rearrange("p (t f) -> p t f", f=P)
    nc.sync.dma_start(out=ep3, in_=pote_in.rearrange("(t p) f -> p t f", p=128))
    nc.scalar.activation(out=ep, in_=ep, func=ACT.Exp)
    ephi = res.tile([128, ET * P], BF, tag="ephi", name="ephi")[:]
    nc.scalar.activation(out=ephi, in_=ep, func=ACT.Copy)
    eplo = res.tile([128, ET * P], BF, tag="eplo", name="eplo")[:]
    nc.vector.tensor_tensor(out=eplo, in0=ep, in1=ephi, op=ALU.subtract)
    ephi3 = ephi.rearrange("p (t f) -> p t f", f=P)
    eplo3 = eplo.rearrange("p (t f) -> p t f", f=P)
    rs2 = res.tile([128, ET], FP, tag="rs2", name="rs2")[:]
    nc.vector.tensor_reduce(out=rs2, in_=ep3, axis=mybir.AxisListType.X, op=ALU.add)
    nc.vector.reciprocal(rs2, rs2)
    lam_ep = res.tile([128, ET], FP, tag="lam_ep", name="lam_ep")[:]
    nc.sync.dma_start(out=lam_ep, in_=lam_in.rearrange("o (t p) -> (o p) t", p=128))
    nc.scalar.activation(out=lam_ep, in_=lam_ep, func=ACT.Sigmoid)
    d2s = res.tile([128, ET], FP, tag="d2s", name="d2s")[:]
    nc.vector.tensor_tensor(out=d2s, in0=lam_ep, in1=rs2, op=ALU.mult)

    # q0 = 1 - sig(slide) - sig(guess), q1 = sig(guess)
    sg = res.tile([128, ET], FP, tag="sg", name="sg")[:]
    nc.sync.dma_start(out=sg, in_=gue_in.rearrange("o (t p) -> (o p) t", p=128))
    nc.scalar.activation(out=sg, in_=sg, func=ACT.Sigmoid)
    ss = res.tile([128, ET], FP, tag="ss", name="ss")[:]
    nc.sync.dma_start(out=ss, in_=sli_in.rearrange("o (t p) -> (o p) t", p=128))
    nc.scalar.activation(out=ss, in_=ss, func=ACT.Sigmoid)
    q0 = res.tile([128, ET], FP, tag="q0", name="q0")[:]
    nc.vector.tensor_scalar(
        out=q0, in0=ss, scalar1=-1.0, scalar2=1.0, op0=ALU.mult, op1=ALU.add
    )
    nc.vector.tensor_tensor(out=q0, in0=q0, in1=sg, op=ALU.subtract)
    nc.sync.dma_start(out=q0h.rearrange("o (t p) -> (o p) t", p=128), in_=q0)
    nc.sync.dma_start(out=q1h.rearrange("o (t p) -> (o p) t", p=128), in_=sg)

    # e = exp(conc_conc_w), resident [c, d] (values {0,5}: exp exact-safe)
    e_mat = []
    for i in range(CT):
        em = res.tile([128, C], FP, tag=f"em{i}", name=f"em{i}")[:]
        nc.sync.dma_start(out=em, in_=cc_in[128 * i : 128 * (i + 1), :])
        nc.scalar.activation(out=em, in_=em, func=ACT.Exp)
        e_mat.append(em)

    # ---------- main streaming pass over exercise tiles ----------
    # per tile t: W_t = sigmoid(ecw)*adj (+rowsum); transposes -> W^T (HBM);
    # matmuls: s/num^T += W_t-block @ M-slice ; Z/V^T += ep_t @ M-slice ;
    # D2L^T transpose -> HBM
    rowsum = res.tile([128, ET], FP, tag="rowsum", name="rowsum")[:]
    wstage = [None] * CT
    with tc.tile_pool(name="pacc", bufs=1, space="PSUM") as pacc:
        psn = [
            pacc.tile([128, 2 * BS], FP, tag=f"psn{i}", name=f"psn{i}")[:]
            for i in range(CT)
        ]
        pvz = pacc.tile([P, 2 * BS], FP, tag="pvz", name="pvz")[:]
        for t in range(ET):
            a_t = stream.tile([128, C], FP, tag="adj", name="adj")[:]
            nc.sync.dma_start(out=a_t, in_=adj_in[128 * t : 128 * (t + 1), :])
            w_t = stream.tile([128, C], FP, tag="w", name="w")[:]
            nc.sync.dma_start(out=w_t, in_=ecw_in[128 * t : 128 * (t + 1), :])
            nc.scalar.activation(out=w_t, in_=w_t, func=ACT.Sigmoid)
            nc.vector.scalar_tensor_tensor(
                out=w_t,
                in0=w_t,
                scalar=1.0,
                in1=a_t,
                op0=ALU.mult,
                op1=ALU.mult,
                accum_out=rowsum[:, t : t + 1],
            )
            whi = stream.tile([128, C], BF, tag="whi", name="whi")[:]
            nc.scalar.activation(out=whi, in_=w_t, func=ACT.Copy)
            wlo = stream.tile([128, C], BF, tag="wlo", name="wlo")[:]
            nc.vector.tensor_tensor(out=wlo, in0=w_t, in1=whi, op=ALU.subtract)
            m_sl = m_all[:, 256 * t : 256 * (t + 1)]
            for ct in range(CT):
                cs = slice(128 * ct, 128 * (ct + 1))
                nc.tensor.matmul(
                    out=psn[ct], lhsT=whi[:, cs], rhs=m_sl,
                    start=(t == 0), stop=False,
                )
                nc.tensor.matmul(
                    out=psn[ct], lhsT=wlo[:, cs], rhs=m_sl,
                    start=False, stop=(t == ET - 1),
                )
                tp = ptr.tile([128, 128], FP, tag="tr", name="tr")[:]
                nc.tensor.transpose(tp, w_t[:, cs], ident)
                wq = t % 4
                wts = wstage[ct]
                if wq == 0:
                    wts = stream.tile([128, 512], FP, tag=f"wts{ct}", name=f"wts{ct}")[:]
                    wstage[ct] = wts
                if (t + ct) % 2 == 0:
                    nc.vector.tensor_copy(
                        out=wts[:, 128 * wq : 128 * (wq + 1)], in_=tp
                    )
                else:
                    nc.scalar.copy(out=wts[:, 128 * wq : 128 * (wq + 1)], in_=tp)
                if wq == 3:
                    nc.sync.dma_start(
                        out=wth[128 * ct : 128 * (ct + 1), 512 * (t // 4) : 512 * (t // 4 + 1)],
                        in_=wts,
                    )
            nc.tensor.matmul(
                out=pvz, lhsT=ephi3[:, t, :], rhs=m_sl,
                start=(t == 0), stop=False,
            )
            nc.tensor.matmul(
                out=pvz, lhsT=eplo3[:, t, :], rhs=m_sl,
                start=False, stop=(t == ET - 1),
            )
            # D2L^T tile: lam*softmax(pote) transposed, bounced to HBM
            d2t = stream.tile([128, P], FP, tag="d2t", name="d2t")[:]
            nc.vector.tensor_scalar(
                out=d2t,
                in0=ep3[:, t, :],
                scalar1=d2s[:, t : t + 1],
                scalar2=None,
                op0=ALU.mult,
            )
            tp2 = ptr.tile([P, 128], FP, tag="tr", name="tr")[:]
            nc.tensor.transpose(tp2, d2t, ident)
            d2st = stream.tile([P, 128], FP, tag="d2st", name="d2st")[:]
            nc.any.tensor_copy(out=d2st, in_=tp2)
            nc.sync.dma_start(out=d2lth[:, 128 * t : 128 * (t + 1)], in_=d2st)

        # c1 = (1 - sig(lambd)) / rowsum
        nc.vector.reciprocal(rowsum, rowsum)
        c1 = res.tile([128, ET], FP, tag="c1", name="c1")[:]
        nc.vector.tensor_scalar(
            out=c1, in0=lam_ep, scalar1=-1.0, scalar2=1.0, op0=ALU.mult, op1=ALU.add
        )
        nc.vector.tensor_tensor(out=c1, in0=c1, in1=rowsum, op=ALU.mult)
        nc.sync.dma_start(out=c1h.rearrange("o (t p) -> (o p) t", p=128), in_=c1)

        # ---------- A branch ----------
        # psn[ct] = [c-block, (cnt|score, b)]: s = cols 0:128, num = 128:256
        # rhsa[ct] columns: [a (128) | mask (128)]
        rhsa = [
            res.tile([128, 256], FP, tag=f"rhsa{i}", name=f"rhsa{i}")[:]
            for i in range(CT)
        ]
        for ct in range(CT):
            s_col = psn[ct][:, 0:BS]
            n_col = psn[ct][:, BS : 2 * BS]
            msl = rhsa[ct][:, 128:256]
            nc.vector.tensor_scalar(
                out=msl, in0=s_col, scalar1=0.0, scalar2=None, op0=ALU.is_gt
            )
            t1 = scr.tile([128, 128], FP, tag="t1", name="t1")[:]
            nc.vector.scalar_tensor_tensor(
                out=t1, in0=msl, scalar=-1.0, in1=s_col, op0=ALU.mult, op1=ALU.add
            )
            nc.vector.tensor_scalar(
                out=t1, in0=t1, scalar1=1.0, scalar2=None, op0=ALU.add
            )
            nc.vector.reciprocal(t1, t1)
            nc.vector.tensor_tensor(
                out=rhsa[ct][:, 0:128], in0=n_col, in1=t1, op=ALU.mult
            )
        # Bm^T = V/Z from pvz = [p, (Z|V, b)]
        bmt = res.tile([P, BS], FP, tag="bmt", name="bmt")[:]
        rz = scr.tile([P, BS], FP, tag="rz", name="rz")[:]
        nc.vector.reciprocal(rz, pvz[:, 0:BS])
        nc.vector.tensor_tensor(out=bmt, in0=pvz[:, BS : 2 * BS], in1=rz, op=ALU.mult)

    at = [res.tile([128, 128], FP, tag=f"at{i}", name=f"at{i}")[:] for i in range(CT)]
    with tc.tile_pool(name="pmm", bufs=2, space="PSUM") as pmm:
        for dt in range(CT):
            pa = pmm.tile([128, 256], FP, tag="pa", name="pa")[:]
            for ct in range(CT):
                nc.tensor.matmul(
                    out=pa,
                    lhsT=e_mat[ct][:, 128 * dt : 128 * (dt + 1)],
                    rhs=rhsa[ct],
                    start=(ct == 0),
                    stop=(ct == CT - 1),
                )
            rec = scr.tile([128, 128], FP, tag="rec", name="rec")[:]
            nc.vector.reciprocal(rec, pa[:, 128:256])
            nc.vector.tensor_tensor(out=at[dt], in0=pa[:, 0:128], in1=rec, op=ALU.mult)
            tp = ptr.tile([128, 128], FP, tag="tr", name="tr")[:]
            nc.tensor.transpose(tp, at[dt], ident)
            asb = stream.tile([128, 128], FP, tag="asb", name="asb")[:]
            nc.any.tensor_copy(out=asb, in_=tp)
            nc.sync.dma_start(out=a_out[:, 128 * dt : 128 * (dt + 1)], in_=asb)

    # ---------- Y assembly ----------
    with tc.tile_pool(name="pyp", bufs=2, space="PSUM") as pyp:
        for ch in range(ECH):
            sl = slice(512 * ch, 512 * (ch + 1))
            cb = rows.tile([128, 512], FP, tag="cb", name="cb")[:]
            nc.sync.dma_start(out=cb, in_=c1h[0:1, sl].to_broadcast((128, 512)))
            q0b = rows.tile([128, 512], FP, tag="q0b", name="q0b")[:]
            nc.sync.dma_start(out=q0b, in_=q0h[0:1, sl].to_broadcast((128, 512)))
            q1b = rows.tile([128, 512], FP, tag="q1b", name="q1b")[:]
            nc.sync.dma_start(out=q1b, in_=q1h[0:1, sl].to_broadcast((128, 512)))
            d2ch = rows.tile([P, 512], FP, tag="d2ch", name="d2ch")[:]
            nc.sync.dma_start(out=d2ch, in_=d2lth[:, sl])
            pyb = pyp.tile([128, 512], FP, tag="pyb", name="pyb")[:]
            nc.tensor.matmul(out=pyb, lhsT=bmt, rhs=d2ch, start=True, stop=True)
            pya = pyp.tile([128, 512], FP, tag="pya", name="pya")[:]
            for ct in range(CT):
                wtc = rows.tile([128, 512], FP, tag=f"wtc{ct}", name=f"wtc{ct}")[:]
                nc.sync.dma_start(out=wtc, in_=wth[128 * ct : 128 * (ct + 1), sl])
                nc.tensor.matmul(
                    out=pya,
                    lhsT=at[ct],
                    rhs=wtc,
                    start=(ct == 0),
                    stop=(ct == CT - 1),
                )
            ysb = ypool.tile([128, 512], FP, tag="ysb", name="ysb")[:]
            nc.vector.tensor_tensor(out=ysb, in0=pya, in1=cb, op=ALU.mult)
            nc.vector.tensor_tensor(out=ysb, in0=ysb, in1=pyb, op=ALU.add)
            nc.vector.tensor_scalar(
                out=ysb,
                in0=ysb,
                scalar1=1e-8,
                scalar2=1.0 - 1e-8,
                op0=ALU.max,
                op1=ALU.min,
            )
            nc.vector.tensor_tensor(out=ysb, in0=ysb, in1=q0b, op=ALU.mult)
            nc.vector.tensor_tensor(out=ysb, in0=ysb, in1=q1b, op=ALU.add)
            nc.sync.dma_start(out=y_out[:, sl], in_=ysb)


def _split_multi_waits(nc):
    """walrus codegen supports a single sync-wait per instruction; peel
    extras into same-engine NoOp carriers placed just before."""
    for fn in nc.m.functions:
        for b in fn.blocks:
            out, changed = [], False
            for inst in b.instructions:
                si = inst.sync_info
                waits = list(si.on_wait) if si and si.on_wait else []
                if len(waits) > 1:
                    for k, w in enumerate(waits[:-1]):
                        nop = mybir.InstNoOp(name=f"{inst.name}-wc{k}", ins=[], outs=[])
                        nop.engine = inst.engine
                        nop.sync_info = type(si)(on_wait=[w], on_update=[])
                        out.append(nop)
                    si.on_wait = waits[-1:]
                    changed = True
                out.append(inst)
            if changed:
                b.instructions = out


_NC_CACHE = None


def _shard(exer_list, score_list, params):
    in_maps = []
    for m in range(NCORES):
        sl = slice(BS * m, BS * (m + 1))
        in_maps.append(
            {"exer_list": exer_list[sl], "score_list": score_list[sl], **params}
        )
    return in_maps


def _make_in_maps(inputs):
    exer_list = np.ascontiguousarray(np.asarray(inputs["exer_list"], np.int32))
    score_list = np.ascontiguousarray(np.asarray(inputs["score_list"], np.float32))
    params = {
        k: np.ascontiguousarray(np.asarray(inputs[k], np.float32))
        for k in (
            "exer_conc_adj",
            "exer_conc_w",
            "conc_conc_w",
            "exer_pote_w",
            "lambd",
            "guess",
            "slide",
        )
    }
    return _shard(exer_list, score_list, params)


def _get_nc():
    global _NC_CACHE
    if _NC_CACHE is None:
        nc = bass.Bass("TRN2", target_bir_lowering=False, debug=False)
        with tile.TileContext(nc) as tc, ExitStack() as ctx:
            _emit(ctx, tc)
        _split_multi_waits(nc)
        _NC_CACHE = nc
    return _NC_CACHE


def kernel(
    exer_list,
    score_list,
    school_feature,
    school_feature_dim_w,
    exer_conc_adj,
    exer_conc_w,
    conc_conc_w,
    exer_pote_w,
    lambd,
    guess,
    slide,
):
    del school_feature, school_feature_dim_w  # unused by the outputs
    nc = _get_nc()
    in_maps = _make_in_maps(
        {
            "exer_list": exer_list,
            "score_list": score_list,
            "exer_conc_adj": exer_conc_adj,
            "exer_conc_w": exer_conc_w,
            "conc_conc_w": conc_conc_w,
            "exer_pote_w": exer_pote_w,
            "lambd": lambd,
            "guess": guess,
            "slide": slide,
        }
    )
    res = run_bass_kernel_spmd(nc, in_maps, list(range(NCORES)))
    a_full = np.concatenate([res.results[m]["A_out"] for m in range(NCORES)], axis=0)
    y_full = np.concatenate([res.results[m]["Y_out"] for m in range(NCORES)], axis=0)
    return a_full.astype(np.float32), y_full.astype(np.float32)


# revision 17
# speedup vs baseline: 1.5885x; 1.5885x over previous
"""Trainium2 Bass kernel for nn_CICDM_Net (cognitive-diagnosis style model).

Data-parallel over 1024 students across 8 NeuronCores (128/core),
parameters replicated. Per core (v2 — matmul-contraction formulation):

  * Count/score matrices M[e, b] = sum_l [exer[b,l]=e] * {1, score[b,l]}
    are built on device: per (student, half-sequence) a bf16 one-hot
    outer product over the split index e = lo*128 + hi (hi = e%128 on
    partitions, lo = e//128 as PSUM free columns), evacuated into
    M_all[e%128, (e//128, {cnt,score}, b)].
  * W = sigmoid(exer_conc_w)*adj is built tile-by-tile (fused row-sum);
    each W tile feeds (a) TensorE transposes -> W^T bounced via HBM for
    the Y_A matmul, (b) dense matmuls lhsT=W-tile x rhs=M-slice
    accumulating s^T,num^T = [c, (s|num, b)] over all 64 e-tiles, and
    (c) lhsT=exp(pote)-tile for Z^T,V^T (potential-softmax sums).
  * a = num/s (masked), A = (a@e)/(mask@e), e = exp(conc_conc_w)
    (softmax shifts dropped: value ranges make exp exact-safe).
  * Y = q0*clip((1-lam)/rowsum*(A@W^T) + lam*(Bm@D2L^T)) + q1, with
    per-exercise coefficient rows DMA-broadcast across partitions.

All matmuls are fp32 (float32r is reduced precision - measured 1.6e-4)
except the one-hot outer products, which are exact in bf16.
"""

import os
import sys

import numpy as np

for _p in ("/opt/trn_rl_repo", "/root/.axon_site/_ro/trn_rl_repo"):
    if os.path.isdir(_p) and _p not in sys.path:
        sys.path.append(_p)

from contextlib import ExitStack

import concourse.bass as bass
import concourse.tile as tile
from concourse import mybir
from concourse.bass_utils import run_bass_kernel_spmd
from concourse.masks import make_identity

FP = mybir.dt.float32
BF = mybir.dt.bfloat16
I32 = mybir.dt.int32
ALU = mybir.AluOpType
ACT = mybir.ActivationFunctionType

E, C, P, L, B = 8192, 512, 32, 256, 1024
NCORES = 8
BS = B // NCORES          # 128 students per core
ET = E // 128             # 64 exercise tiles
CT = C // 128             # 4 concept tiles
ECH = E // 512            # 16 output chunks


def _emit(ctx: ExitStack, tc: tile.TileContext):
    nc = tc.nc

    x_in = nc.dram_tensor("exer_list", (BS, L), I32, kind="ExternalInput").ap()
    sc_in = nc.dram_tensor("score_list", (BS, L), FP, kind="ExternalInput").ap()
    adj_in = nc.dram_tensor("exer_conc_adj", (E, C), FP, kind="ExternalInput").ap()
    ecw_in = nc.dram_tensor("exer_conc_w", (E, C), FP, kind="ExternalInput").ap()
    cc_in = nc.dram_tensor("conc_conc_w", (C, C), FP, kind="ExternalInput").ap()
    pote_in = nc.dram_tensor("exer_pote_w", (E, P), FP, kind="ExternalInput").ap()
    lam_in = nc.dram_tensor("lambd", (1, E), FP, kind="ExternalInput").ap()
    gue_in = nc.dram_tensor("guess", (1, E), FP, kind="ExternalInput").ap()
    sli_in = nc.dram_tensor("slide", (1, E), FP, kind="ExternalInput").ap()
    a_out = nc.dram_tensor("A_out", (BS, C), FP, kind="ExternalOutput").ap()
    y_out = nc.dram_tensor("Y_out", (BS, E), FP, kind="ExternalOutput").ap()

    dram = ctx.enter_context(tc.tile_pool(name="dram", bufs=1, space="DRAM"))
    wth = dram.tile([C, E], FP, tag="wth", name="wth")[:]
    d2lth = dram.tile([P, E], FP, tag="d2lth", name="d2lth")[:]
    c1h = dram.tile([1, E], FP, tag="c1h", name="c1h")[:]
    q0h = dram.tile([1, E], FP, tag="q0h", name="q0h")[:]
    q1h = dram.tile([1, E], FP, tag="q1h", name="q1h")[:]

    res = ctx.enter_context(tc.tile_pool(name="res", bufs=1))
    scr = ctx.enter_context(tc.tile_pool(name="scr", bufs=2))
    stream = ctx.enter_context(tc.tile_pool(name="stream", bufs=4))
    oh = ctx.enter_context(tc.tile_pool(name="oh", bufs=4))
    ypool = ctx.enter_context(tc.tile_pool(name="ypool", bufs=2))
    rows = ctx.enter_context(tc.tile_pool(name="rows", bufs=2))
    ptr = ctx.enter_context(tc.tile_pool(name="ptr", bufs=2, space="PSUM"))

    # ---------- small/shared prep ----------
    ident = res.tile([128, 128], FP, tag="ident", name="ident")[:]
    make_identity(nc, ident)
    iota128 = res.tile([128, 128], FP, tag="iota128", name="iota128")[:]
    it128 = scr.tile([128, 128], I32, tag="it128", name="it128")[:]
    nc.gpsimd.iota(it128, pattern=[[1, 128]], base=0, channel_multiplier=0)
    nc.vector.tensor_copy(out=iota128, in_=it128)
    iota64 = res.tile([128, 64], FP, tag="iota64", name="iota64")[:]
    nc.vector.tensor_copy(out=iota64, in_=it128[:, 0:64])

    x_sb = stream.tile([BS, L], I32, tag="x", name="x")[:]
    nc.sync.dma_start(out=x_sb, in_=x_in)
    sc_sb = stream.tile([BS, L], FP, tag="sc", name="sc")[:]
    nc.sync.dma_start(out=sc_sb, in_=sc_in)

    # transposed per-half-sequence tiles: hi = x%128, lo = x//128 as f32,
    # score^T; all [l%128, b]
    xf = stream.tile([BS, L], FP, tag="xf", name="xf")[:]
    nc.vector.tensor_copy(out=xf, in_=x_sb)
    hiT, loT, scT = [], [], []
    for j in range(2):
        tpx = ptr.tile([128, 128], FP, tag="tr", name="tr")[:]
        nc.tensor.transpose(tpx, xf[:, 128 * j : 128 * (j + 1)], ident)
        xti = scr.tile([128, 128], I32, tag="xti", name="xti")[:]
        nc.vector.tensor_copy(out=xti, in_=tpx)
        hii = scr.tile([128, 128], I32, tag="hii", name="hii")[:]
        nc.vector.tensor_scalar(
            out=hii, in0=xti, scalar1=127, scalar2=None, op0=ALU.bitwise_and
        )
        hif = res.tile([128, 128], FP, tag=f"hif{j}", name=f"hif{j}")[:]
        nc.vector.tensor_copy(out=hif, in_=hii)
        hiT.append(hif)
        loi = scr.tile([128, 128], I32, tag="loi", name="loi")[:]
        nc.vector.tensor_scalar(
            out=loi, in0=xti, scalar1=7, scalar2=None, op0=ALU.logical_shift_right
        )
        lof = res.tile([128, 128], FP, tag=f"lof{j}", name=f"lof{j}")[:]
        nc.vector.tensor_copy(out=lof, in_=loi)
        loT.append(lof)
        tps = ptr.tile([128, 128], FP, tag="tr", name="tr")[:]
        nc.tensor.transpose(tps, sc_sb[:, 128 * j : 128 * (j + 1)], ident)
        sct = res.tile([128, 128], FP, tag=f"sct{j}", name=f"sct{j}")[:]
        nc.vector.tensor_copy(out=sct, in_=tps)
        scT.append(sct)

    # ---------- M build: one-hot outer products, 1 student per PSUM ----------
    # M_all[p, (t, half, b)] = M[e = t*128 + p, b], half 0 = count, 1 = score
    m_all = res.tile([128, ET * 2 * BS], BF, tag="m_all", name="m_all")[:]
    m4 = m_all.rearrange("p (t h b) -> p t h b", h=2, b=BS)
    with tc.tile_pool(name="pmb", bufs=2, space="PSUM") as pmb:
        for b in range(BS):
            pm = pmb.tile([128, 128], FP, tag="pm", name="pm")[:]
            for j in range(2):
                ohh = oh.tile([128, 128], BF, tag="ohh", name="ohh")[:]
                nc.vector.tensor_scalar(
                    out=ohh,
                    in0=iota128,
                    scalar1=hiT[j][:, b : b + 1],
                    scalar2=None,
                    op0=ALU.is_equal,
                )
                rhsb = oh.tile([128, 128], BF, tag="rhsb", name="rhsb")[:]
                nc.vector.tensor_scalar(
                    out=rhsb[:, 0:64],
                    in0=iota64,
                    scalar1=loT[j][:, b : b + 1],
                    scalar2=None,
                    op0=ALU.is_equal,
                )
                nc.scalar.activation(
                    out=rhsb[:, 64:128],
                    in_=rhsb[:, 0:64],
                    func=ACT.Copy,
                    scale=scT[j][:, b : b + 1],
                )
                nc.tensor.matmul(
                    out=pm, lhsT=ohh, rhs=rhsb, start=(j == 0), stop=(j == 1)
                )
            nc.vector.tensor_copy(out=m4[:, :, 0, b], in_=pm[:, 0:64])
            nc.vector.tensor_copy(out=m4[:, :, 1, b], in_=pm[:, 64:128])

    # ---------- exp(pote), D2 scale, coefficient rows ----------
    # ep layout: [p, t, f] with exercise e = 128*t + p
    ep = res.tile([128, ET * P], FP, tag="ep", name="ep")[:]
    ep3 = ep.rearrange("p (t f) -> p t f", f=P)
    nc.sync.dma_start(out=ep3, in_=pote_in.rearrange("(t p) f -> p t f", p=128))
    nc.scalar.activation(out=ep, in_=ep, func=ACT.Exp)
    ephi = res.tile([128, ET * P], BF, tag="ephi", name="ephi")[:]
    nc.scalar.activation(out=ephi, in_=ep, func=ACT.Copy)
    eplo = res.tile([128, ET * P], BF, tag="eplo", name="eplo")[:]
    nc.vector.tensor_tensor(out=eplo, in0=ep, in1=ephi, op=ALU.subtract)
    ephi3 = ephi.rearrange("p (t f) -> p t f", f=P)
    eplo3 = eplo.rearrange("p (t f) -> p t f", f=P)
    rs2 = res.tile([128, ET], FP, tag="rs2", name="rs2")[:]
    nc.vector.tensor_reduce(out=rs2, in_=ep3, axis=mybir.AxisListType.X, op=ALU.add)
    nc.vector.reciprocal(rs2, rs2)
    lam_ep = res.tile([128, ET], FP, tag="lam_ep", name="lam_ep")[:]
    nc.sync.dma_start(out=lam_ep, in_=lam_in.rearrange("o (t p) -> (o p) t", p=128))
    nc.scalar.activation(out=lam_ep, in_=lam_ep, func=ACT.Sigmoid)
    d2s = res.tile([128, ET], FP, tag="d2s", name="d2s")[:]
    nc.vector.tensor_tensor(out=d2s, in0=lam_ep, in1=rs2, op=ALU.mult)

    # q0 = 1 - sig(slide) - sig(guess), q1 = sig(guess)
    sg = res.tile([128, ET], FP, tag="sg", name="sg")[:]
    nc.sync.dma_start(out=sg, in_=gue_in.rearrange("o (t p) -> (o p) t", p=128))
    nc.scalar.activation(out=sg, in_=sg, func=ACT.Sigmoid)
    ss = res.tile([128, ET], FP, tag="ss", name="ss")[:]
    nc.sync.dma_start(out=ss, in_=sli_in.rearrange("o (t p) -> (o p) t", p=128))
    nc.scalar.activation(out=ss, in_=ss, func=ACT.Sigmoid)
    q0 = res.tile([128, ET], FP, tag="q0", name="q0")[:]
    nc.vector.tensor_scalar(
        out=q0, in0=ss, scalar1=-1.0, scalar2=1.0, op0=ALU.mult, op1=ALU.add
    )
    nc.vector.tensor_tensor(out=q0, in0=q0, in1=sg, op=ALU.subtract)
    nc.sync.dma_start(out=q0h.rearrange("o (t p) -> (o p) t", p=128), in_=q0)
    nc.sync.dma_start(out=q1h.rearrange("o (t p) -> (o p) t", p=128), in_=sg)

    # e = exp(conc_conc_w), resident [c, d] (values {0,5}: exp exact-safe)
    e_mat = []
    for i in range(CT):
        em = res.tile([128, C], FP, tag=f"em{i}", name=f"em{i}")[:]
        nc.sync.dma_start(out=em, in_=cc_in[128 * i : 128 * (i + 1), :])
        nc.scalar.activation(out=em, in_=em, func=ACT.Exp)
        e_mat.append(em)

    # ---------- main streaming pass over exercise tiles ----------
    # per tile t: W_t = sigmoid(ecw)*adj (+rowsum); transposes -> W^T (HBM);
    # matmuls: s/num^T += W_t-block @ M-slice ; Z/V^T += ep_t @ M-slice ;
    # D2L^T transpose -> HBM
    rowsum = res.tile([128, ET], FP, tag="rowsum", name="rowsum")[:]
    wstage = [None] * CT
    with tc.tile_pool(name="pacc", bufs=1, space="PSUM") as pacc:
        psn = [
            pacc.tile([128, 2 * BS], FP, tag=f"psn{i}", name=f"psn{i}")[:]
            for i in range(CT)
        ]
        pvz = pacc.tile([P, 2 * BS], FP, tag="pvz", name="pvz")[:]
        for t in range(ET):
            a_t = stream.tile([128, C], FP, tag="adj", name="adj")[:]
            nc.sync.dma_start(out=a_t, in_=adj_in[128 * t : 128 * (t + 1), :])
            w_t = stream.tile([128, C], FP, tag="w", name="w")[:]
            nc.sync.dma_start(out=w_t, in_=ecw_in[128 * t : 128 * (t + 1), :])
            nc.scalar.activation(out=w_t, in_=w_t, func=ACT.Sigmoid)
            nc.vector.scalar_tensor_tensor(
                out=w_t,
                in0=w_t,
                scalar=1.0,
                in1=a_t,
                op0=ALU.mult,
                op1=ALU.mult,
                accum_out=rowsum[:, t : t + 1],
            )
            whi = stream.tile([128, C], BF, tag="whi", name="whi")[:]
            nc.scalar.activation(out=whi, in_=w_t, func=ACT.Copy)
            wlo = stream.tile([128, C], BF, tag="wlo", name="wlo")[:]
            nc.vector.tensor_tensor(out=wlo, in0=w_t, in1=whi, op=ALU.subtract)
            m_sl = m_all[:, 256 * t : 256 * (t + 1)]
            for ct in range(CT):
                cs = slice(128 * ct, 128 * (ct + 1))
                nc.tensor.matmul(
                    out=psn[ct], lhsT=whi[:, cs], rhs=m_sl,
                    start=(t == 0), stop=False,
                )
                nc.tensor.matmul(
                    out=psn[ct], lhsT=wlo[:, cs], rhs=m_sl,
                    start=False, stop=(t == ET - 1),
                )
                tp = ptr.tile([128, 128], FP, tag="tr", name="tr")[:]
                nc.tensor.transpose(tp, w_t[:, cs], ident)
                wq = t % 4
                wts = wstage[ct]
                if wq == 0:
                    wts = stream.tile([128, 512], FP, tag=f"wts{ct}", name=f"wts{ct}")[:]
                    wstage[ct] = wts
                if (t + ct) % 2 == 0:
                    nc.vector.tensor_copy(
                        out=wts[:, 128 * wq : 128 * (wq + 1)], in_=tp
                    )
                else:
                    nc.scalar.copy(out=wts[:, 128 * wq : 128 * (wq + 1)], in_=tp)
                if wq == 3:
                    nc.sync.dma_start(
                        out=wth[128 * ct : 128 * (ct + 1), 512 * (t // 4) : 512 * (t // 4 + 1)],
                        in_=wts,
                    )
            nc.tensor.matmul(
                out=pvz, lhsT=ephi3[:, t, :], rhs=m_sl,
                start=(t == 0), stop=False,
            )
            nc.tensor.matmul(
                out=pvz, lhsT=eplo3[:, t, :], rhs=m_sl,
                start=False, stop=(t == ET - 1),
            )
            # D2L^T tile: lam*softmax(pote) transposed, bounced to HBM
            d2t = stream.tile([128, P], FP, tag="d2t", name="d2t")[:]
            nc.vector.tensor_scalar(
                out=d2t,
                in0=ep3[:, t, :],
                scalar1=d2s[:, t : t + 1],
                scalar2=None,
                op0=ALU.mult,
            )
            tp2 = ptr.tile([P, 128], FP, tag="tr", name="tr")[:]
            nc.tensor.transpose(tp2, d2t, ident)
            d2st = stream.tile([P, 128], FP, tag="d2st", name="d2st")[:]
            nc.any.tensor_copy(out=d2st, in_=tp2)
            nc.sync.dma_start(out=d2lth[:, 128 * t : 128 * (t + 1)], in_=d2st)

        # c1 = (1 - sig(lambd)) / rowsum
        nc.vector.reciprocal(rowsum, rowsum)
        c1 = res.tile([128, ET], FP, tag="c1", name="c1")[:]
        nc.vector.tensor_scalar(
            out=c1, in0=lam_ep, scalar1=-1.0, scalar2=1.0, op0=ALU.mult, op1=ALU.add
        )
        nc.vector.tensor_tensor(out=c1, in0=c1, in1=rowsum, op=ALU.mult)
        nc.sync.dma_start(out=c1h.rearrange("o (t p) -> (o p) t", p=128), in_=c1)

        # ---------- A branch ----------
        # psn[ct] = [c-block, (cnt|score, b)]: s = cols 0:128, num = 128:256
        # rhsa[ct] columns: [a (128) | mask (128)]
        rhsa = [
            res.tile([128, 256], FP, tag=f"rhsa{i}", name=f"rhsa{i}")[:]
            for i in range(CT)
        ]
        for ct in range(CT):
            s_col = psn[ct][:, 0:BS]
            n_col = psn[ct][:, BS : 2 * BS]
            msl = rhsa[ct][:, 128:256]
            nc.vector.tensor_scalar(
                out=msl, in0=s_col, scalar1=0.0, scalar2=None, op0=ALU.is_gt
            )
            t1 = scr.tile([128, 128], FP, tag="t1", name="t1")[:]
            nc.vector.scalar_tensor_tensor(
                out=t1, in0=msl, scalar=-1.0, in1=s_col, op0=ALU.mult, op1=ALU.add
            )
            nc.vector.tensor_scalar(
                out=t1, in0=t1, scalar1=1.0, scalar2=None, op0=ALU.add
            )
            nc.vector.reciprocal(t1, t1)
            nc.vector.tensor_tensor(
                out=rhsa[ct][:, 0:128], in0=n_col, in1=t1, op=ALU.mult
            )
        # Bm^T = V/Z from pvz = [p, (Z|V, b)]
        bmt = res.tile([P, BS], FP, tag="bmt", name="bmt")[:]
        rz = scr.tile([P, BS], FP, tag="rz", name="rz")[:]
        nc.vector.reciprocal(rz, pvz[:, 0:BS])
        nc.vector.tensor_tensor(out=bmt, in0=pvz[:, BS : 2 * BS], in1=rz, op=ALU.mult)

    at = [res.tile([128, 128], FP, tag=f"at{i}", name=f"at{i}")[:] for i in range(CT)]
    with tc.tile_pool(name="pmm", bufs=2, space="PSUM") as pmm:
        for dt in range(CT):
            pa = pmm.tile([128, 256], FP, tag="pa", name="pa")[:]
            for ct in range(CT):
                nc.tensor.matmul(
                    out=pa,
                    lhsT=e_mat[ct][:, 128 * dt : 128 * (dt + 1)],
                    rhs=rhsa[ct],
                    start=(ct == 0),
                    stop=(ct == CT - 1),
                )
            rec = scr.tile([128, 128], FP, tag="rec", name="rec")[:]
            nc.vector.reciprocal(rec, pa[:, 128:256])
            nc.vector.tensor_tensor(out=at[dt], in0=pa[:, 0:128], in1=rec, op=ALU.mult)
            tp = ptr.tile([128, 128], FP, tag="tr", name="tr")[:]
            nc.tensor.transpose(tp, at[dt], ident)
            asb = stream.tile([128, 128], FP, tag="asb", name="asb")[:]
            nc.any.tensor_copy(out=asb, in_=tp)
            nc.sync.dma_start(out=a_out[:, 128 * dt : 128 * (dt + 1)], in_=asb)

    # ---------- Y assembly ----------
    with tc.tile_pool(name="pyp", bufs=2, space="PSUM") as pyp:
        for ch in range(ECH):
            sl = slice(512 * ch, 512 * (ch + 1))
            cb = rows.tile([128, 512], FP, tag="cb", name="cb")[:]
            nc.sync.dma_start(out=cb, in_=c1h[0:1, sl].to_broadcast((128, 512)))
            q0b = rows.tile([128, 512], FP, tag="q0b", name="q0b")[:]
            nc.sync.dma_start(out=q0b, in_=q0h[0:1, sl].to_broadcast((128, 512)))
            q1b = rows.tile([128, 512], FP, tag="q1b", name="q1b")[:]
            nc.sync.dma_start(out=q1b, in_=q1h[0:1, sl].to_broadcast((128, 512)))
            d2ch = rows.tile([P, 512], FP, tag="d2ch", name="d2ch")[:]
            nc.sync.dma_start(out=d2ch, in_=d2lth[:, sl])
            pyb = pyp.tile([128, 512], FP, tag="pyb", name="pyb")[:]
            nc.tensor.matmul(out=pyb, lhsT=bmt, rhs=d2ch, start=True, stop=True)
            pya = pyp.tile([128, 512], FP, tag="pya", name="pya")[:]
            for ct in range(CT):
                wtc = rows.tile([128, 512], FP, tag=f"wtc{ct}", name=f"wtc{ct}")[:]
                nc.sync.dma_start(out=wtc, in_=wth[128 * ct : 128 * (ct + 1), sl])
                nc.tensor.matmul(
                    out=pya,
                    lhsT=at[ct],
                    rhs=wtc,
                    start=(ct == 0),
                    stop=(ct == CT - 1),
                )
            ysb = ypool.tile([128, 512], FP, tag="ysb", name="ysb")[:]
            nc.vector.tensor_tensor(out=ysb, in0=pya, in1=cb, op=ALU.mult)
            nc.vector.tensor_tensor(out=ysb, in0=ysb, in1=pyb, op=ALU.add)
            nc.vector.tensor_scalar(
                out=ysb,
                in0=ysb,
                scalar1=1e-8,
                scalar2=1.0 - 1e-8,
                op0=ALU.max,
                op1=ALU.min,
            )
            nc.vector.tensor_tensor(out=ysb, in0=ysb, in1=q0b, op=ALU.mult)
            nc.vector.tensor_tensor(out=ysb, in0=ysb, in1=q1b, op=ALU.add)
            nc.sync.dma_start(out=y_out[:, sl], in_=ysb)


def _split_multi_waits(nc):
    """walrus codegen supports a single sync-wait per instruction; peel
    extras into same-engine NoOp carriers placed just before."""
    for fn in nc.m.functions:
        for b in fn.blocks:
            out, changed = [], False
            for inst in b.instructions:
                si = inst.sync_info
                waits = list(si.on_wait) if si and si.on_wait else []
                if len(waits) > 1:
                    for k, w in enumerate(waits[:-1]):
                        nop = mybir.InstNoOp(name=f"{inst.name}-wc{k}", ins=[], outs=[])
                        nop.engine = inst.engine
                        nop.sync_info = type(si)(on_wait=[w], on_update=[])
                        out.append(nop)
                    si.on_wait = waits[-1:]
                    changed = True
                out.append(inst)
            if changed:
                b.instructions = out


_NC_CACHE = None


def _shard(exer_list, score_list, params):
    in_maps = []
    for m in range(NCORES):
        sl = slice(BS * m, BS * (m + 1))
        in_maps.append(
            {"exer_list": exer_list[sl], "score_list": score_list[sl], **params}
        )
    return in_maps


def _make_in_maps(inputs):
    exer_list = np.ascontiguousarray(np.asarray(inputs["exer_list"], np.int32))
    score_list = np.ascontiguousarray(np.asarray(inputs["score_list"], np.float32))
    params = {
        k: np.ascontiguousarray(np.asarray(inputs[k], np.float32))
        for k in (
            "exer_conc_adj",
            "exer_conc_w",
            "conc_conc_w",
            "exer_pote_w",
            "lambd",
            "guess",
            "slide",
        )
    }
    return _shard(exer_list, score_list, params)


def _get_nc():
    global _NC_CACHE
    if _NC_CACHE is None:
        nc = bass.Bass("TRN2", target_bir_lowering=False, debug=False)
        with tile.TileContext(nc) as tc, ExitStack() as ctx:
            _emit(ctx, tc)
        _split_multi_waits(nc)
        _NC_CACHE = nc
    return _NC_CACHE


def kernel(
    exer_list,
    score_list,
    school_feature,
    school_feature_dim_w,
    exer_conc_adj,
    exer_conc_w,
    conc_conc_w,
    exer_pote_w,
    lambd,
    guess,
    slide,
):
    del school_feature, school_feature_dim_w  # unused by the outputs
    nc = _get_nc()
    in_maps = _make_in_maps(
        {
            "exer_list": exer_list,
            "score_list": score_list,
            "exer_conc_adj": exer_conc_adj,
            "exer_conc_w": exer_conc_w,
            "conc_conc_w": conc_conc_w,
            "exer_pote_w": exer_pote_w,
            "lambd": lambd,
            "guess": guess,
            "slide": slide,
        }
    )
    res = run_bass_kernel_spmd(nc, in_maps, list(range(NCORES)))
    a_full = np.concatenate([res.results[m]["A_out"] for m in range(NCORES)], axis=0)
    y_full = np.concatenate([res.results[m]["Y_out"] for m in range(NCORES)], axis=0)
    return a_full.astype(np.float32), y_full.astype(np.float32)
